# revision 49
# baseline (speedup 1.0000x reference)
"""AnchorSet2NodeMPNN Trainium2 kernel (8 NeuronCores, graph-parallel).

Each core handles one graph (N=384 nodes, A=64 anchors, H=256, E=64).

Algorithmic structure (per core):
  d^2[n,a] = |nx|^2 + |ax|^2 - 2 nx.ax           (rank-5 K=5 matmul)
  t'[n,a]  = sqrt(d^2) * (0.1/sigma)              (ACT sqrt; staged to DRAM)
  Layer 1 is factored: pair @ W1a = nf@W1a[:H] + af@W1a[H:2H] + rbf@W1a[2H:]
    - NA = nf@W1a[:H]    per-node   (computed once)
    - AF' = af@W1a[H:2H] + b1a  per-anchor (computed once)
    All three terms are fused into ONE fp8 DoubleRow matmul with K=256:
      ko=0: rows 0-63 = 64*W1r (rbf weights), rows 64-127 = 64*AF' with
            anchor-indicator rhs; ko=1: 64*NA with node-indicator rhs.
    Per chunk (8 nodes x 64 anchors) the rhs is a per-variant static tile
    [rbf | mast_a | mast_n] where only the rbf quarter is rewritten.
  Layer 2 per-pair dense in fp8 DoubleRow (K=512 as 2 passes of 256),
    weights scaled x64 to stay in fp8-normal range; relu rescales by 1/64.
  Layer 3 commutes with the anchor-mean: upd = (sum_a h2) @ W1c / 64^2 + b1c
    (h2 is kept 64x-scaled so relu biases stay per-partition on ACT/DVE).
  Anchor-sum: two halving adds on GPSIMD (Pool) + short DVE tensor_reduce.
  LayerNorms run feature-transposed; cross-partition sums use an all-ones
  fp32 PE matmul (result broadcast to every partition); 1/sqrt(v) is
  exp(-0.5*ln(v)) (no DVE-reciprocal roundtrip).

Engine balance per chunk: PE 12 DR matmuls; ACT relu-h1(half)+relu-h2(2/4)
+rbf square/exp; DVE relu-h1(half)+relu-h2(2/4)+final reduce; Pool halvings.
Activation float-biases are passed as a preloaded zero column, never 0.0
immediates (a float bias materializes a const AP whose DMA would serialize
the ACT queue behind the weight-load flood).

fp8 quantization error (vs fp32 reference) is ~2.2e-3 overall; anchor-mean
averaging suppresses activation-quantization noise.
"""
import numpy as np
import ml_dtypes

import concourse.bass as bass
import concourse.mybir as mybir
import concourse.tile as tile
from concourse import bacc
from concourse.bass_utils import run_bass_kernel_spmd

F32 = mybir.dt.float32
BF16 = mybir.dt.bfloat16
F8 = mybir.dt.float8e4
AF = mybir.ActivationFunctionType
DR = mybir.MatmulPerfMode.DoubleRow

B, N, A, H, E = 8, 384, 64, 256, 64
RBF_D_MAX = 20.0
SIGMA = RBF_D_MAX / E                    # 0.3125
MU = np.linspace(0.0, RBF_D_MAX, E).astype(np.float32)
NC_CHUNKS = 48                           # sub-chunks of 8 nodes x 64 anchors
D2_SCALE = 0.01 / SIGMA**2               # t' = sqrt(d2 * D2_SCALE) = (d/10)/sigma
FS = 64.0                                # fp8 weight scale


def _consts():
    # mast_a[a', j] = 1 if a' == j % 64
    mast_a = np.zeros((64, 512), np.float32)
    for j in range(512):
        mast_a[j % 64, j] = 1.0
    # mast_n[k, 512*(8q+s) + j] = 1 if k == 64q + 8s + j//64
    mast_n = np.zeros((128, 8192), np.float32)
    for q in range(2):
        for s in range(8):
            for j in range(512):
                k = 64 * q + 8 * s + j // 64
                mast_n[k, 4096 * q + 512 * s + j] = 1.0
    ident = np.eye(128, dtype=np.float32)
    negmusig = np.tile(-(MU / SIGMA).astype(np.float32), 2)
    ones64 = np.ones((1, 64), np.float32)
    return dict(
        c_mast_a=mast_a.astype(ml_dtypes.float8_e4m3),
        c_mast_n=mast_n.astype(ml_dtypes.float8_e4m3),
        c_ident=ident,
        c_negmusig=negmusig,
        c_ones64=ones64.astype(ml_dtypes.bfloat16),
    )


def _pack_biases(inputs):
    """Column-pack per-feature vectors: [128, 24] f32, layout-only."""
    cols = []
    for k, n in (("b1b", 4), ("b1c", 2), ("b2a", 4), ("b2b", 4), ("b2c", 2),
                 ("ln1_g", 2), ("ln1_b", 2), ("ln2_g", 2), ("ln2_b", 2)):
        v = np.asarray(inputs[k], np.float32)
        cols.append(v.reshape(n, 128).T)
    b64 = 64.0 * np.asarray(inputs["b1b"], np.float32)
    cols.append(b64.reshape(4, 128).T)
    cols.append(-b64.reshape(4, 128).T)
    return np.ascontiguousarray(np.concatenate(cols, axis=1))


def _build():
    nc = bacc.Bacc("TRN2", target_bir_lowering=False, debug=False)

    # ---- parameters ----
    p_nx = nc.declare_dram_parameter("node_x", [N, 3], F32, isOutput=False)
    p_ax = nc.declare_dram_parameter("anchor_x", [A, 3], F32, isOutput=False)
    p_nf = nc.declare_dram_parameter("node_features", [N, H], F32, isOutput=False)
    p_af = nc.declare_dram_parameter("anchor_features", [A, H], F32, isOutput=False)
    p_mask = nc.declare_dram_parameter("node_mask", [N], F32, isOutput=False)
    p_w1a = nc.declare_dram_parameter("W1a", [2 * H + E, 512], F32, isOutput=False)
    p_b1a = nc.declare_dram_parameter("b1a", [512], F32, isOutput=False)
    p_w1b = nc.declare_dram_parameter("W1b", [512, 512], F32, isOutput=False)
    p_b1b = nc.declare_dram_parameter("b1b", [512], F32, isOutput=False)
    p_w1c = nc.declare_dram_parameter("W1c", [512, H], F32, isOutput=False)
    p_b1c = nc.declare_dram_parameter("b1c", [H], F32, isOutput=False)
    p_ln1g = nc.declare_dram_parameter("ln1_g", [H], F32, isOutput=False)
    p_ln1b = nc.declare_dram_parameter("ln1_b", [H], F32, isOutput=False)
    p_w2a = nc.declare_dram_parameter("W2a", [H, 512], F32, isOutput=False)
    p_b2a = nc.declare_dram_parameter("b2a", [512], F32, isOutput=False)
    p_w2b = nc.declare_dram_parameter("W2b", [512, 512], F32, isOutput=False)
    p_b2b = nc.declare_dram_parameter("b2b", [512], F32, isOutput=False)
    p_w2c = nc.declare_dram_parameter("W2c", [512, H], F32, isOutput=False)
    p_b2c = nc.declare_dram_parameter("b2c", [H], F32, isOutput=False)
    p_ln2g = nc.declare_dram_parameter("ln2_g", [H], F32, isOutput=False)
    p_ln2b = nc.declare_dram_parameter("ln2_b", [H], F32, isOutput=False)
    c_mast_a = nc.declare_dram_parameter("c_mast_a", [64, 512], F8, isOutput=False)
    c_mast_n = nc.declare_dram_parameter("c_mast_n", [128, 8192], F8, isOutput=False)
    c_biases = nc.declare_dram_parameter("c_biases", [128, 32], F32, isOutput=False)
    c_ident = nc.declare_dram_parameter("c_ident", [128, 128], F32, isOutput=False)
    c_negmusig = nc.declare_dram_parameter("c_negmusig", [2 * E], F32, isOutput=False)
    c_ones64 = nc.declare_dram_parameter("c_ones64", [1, 64], BF16, isOutput=False)
    p_out = nc.declare_dram_parameter("out", [N, H], F32, isOutput=True)

    t_dram3 = nc.dram_tensor("t_scratch", [128, 3 * A], F32)

    with tile.TileContext(nc) as tc:
        with (
            tc.tile_pool(name="wp", bufs=1) as wp,
            tc.tile_pool(name="psA", bufs=2, space="PSUM") as psA,
            tc.tile_pool(name="psB", bufs=3, space="PSUM") as psB,
            tc.tile_pool(name="psC", bufs=1, space="PSUM") as psC,
            tc.tile_pool(name="tbp", bufs=3) as tbp,
            tc.tile_pool(name="qp", bufs=2) as qp,
            tc.tile_pool(name="h1p", bufs=4) as h1p,
            tc.tile_pool(name="h2p", bufs=6) as h2p,
            tc.tile_pool(name="redp", bufs=4) as redp,
            tc.tile_pool(name="outp", bufs=2) as outp,
        ):
            dma = nc.sync.dma_start

            # ================= phase 0: loads =================
            nx_sb = [wp.tile([128, 3], F32, tag=f"nx{r}", name=f"nx{r}")
                     for r in range(3)]
            for r in range(3):
                dma(nx_sb[r], p_nx[128 * r:128 * r + 128, :])
            ax_sb = wp.tile([64, 3], F32)
            dma(ax_sb, p_ax[:])
            ident = wp.tile([128, 128], F32); dma(ident, c_ident[:])
            negmu = wp.tile([128, 1], F32)
            dma(negmu, c_negmusig[:].rearrange("(p o) -> p o", o=1))
            w1nf = wp.tile([128, 1024], F32)
            nc.gpsimd.dma_start(w1nf.rearrange("p (k f) -> p k f", k=2), p_w1a[0:256, :].rearrange("(k p) f -> p k f", p=128))
            w1af = wp.tile([128, 1024], F32)
            nc.gpsimd.dma_start(w1af.rearrange("p (k f) -> p k f", k=2), p_w1a[256:512, :].rearrange("(k p) f -> p k f", p=128))
            w1r = wp.tile([64, 512], F32)
            dma(w1r, p_w1a[512:576, :])
            nf_sb = [wp.tile([128, 256], F32, tag=f"nfsb{r}", name=f"nfsb{r}")
                     for r in range(3)]
            for r in range(3):
                nc.gpsimd.dma_start(nf_sb[r], p_nf[128 * r:128 * r + 128, :])
            # 16 variant blocks in one parent tile, each 1024 cols:
            # [rbf (dynamic) | mast_a rows 64-127 | mast_n variant]
            vtp = wp.tile([128, 16384], F8)
            vt = [vtp[:, 1024 * v:1024 * v + 1024] for v in range(16)]
            nc.gpsimd.dma_start(
                out=vtp[64:128, :].rearrange("p (v x) -> p v x", x=1024)[:, :, 0:512],
                in_=bass.AP(tensor=c_mast_a[:].tensor, offset=0,
                            ap=[[512, 64], [0, 16], [1, 512]]),
            )
            nc.gpsimd.dma_start(
                out=vtp.rearrange("p (v x) -> p v x", x=1024)[:, :, 512:1024],
                in_=c_mast_n[:].rearrange("p (v x) -> p v x", x=512),
            )
            b1a_row = wp.tile([1, 512], F32)
            dma(b1a_row, p_b1a[:].rearrange("(o f) -> o f", o=1))
            ones64 = wp.tile([1, 64], BF16); dma(ones64, c_ones64[:])
            af_sb = wp.tile([64, 256], F32)
            nc.gpsimd.dma_start(af_sb, p_af[:])
            w1b = wp.tile([128, 2048], F32)
            nc.gpsimd.dma_start(w1b.rearrange("p (k f) -> p k f", k=4), p_w1b[:].rearrange("(k p) f -> p k f", p=128))

            # packed bias columns
            bias_pack = wp.tile([128, 32], F32)
            dma(bias_pack, c_biases[:])
            off = [0]
            def bp(n):
                t = bias_pack[:, off[0]:off[0] + n]
                off[0] += n
                return t
            b1b_c = bp(4); b1c_c = bp(2); b2a_c = bp(4); b2b_c = bp(4)
            b2c_c = bp(2); ln1g_c = bp(2); ln1b_c = bp(2); ln2g_c = bp(2)
            ln2b_c = bp(2); b1b64_c = bp(4); nb1b64_c = bp(4)
            b1a_rb = wp.tile([1, 512], BF16); nc.vector.tensor_copy(b1a_rb, b1a_row)
            eps_c = wp.tile([128, 1], F32)
            nc.vector.memset(eps_c, 1e-5)
            zcol = wp.tile([128, 1], F32)
            nc.vector.memset(zcol, 0.0)
            ones_f = wp.tile([128, 128], F32)
            nc.vector.memset(ones_f, 1.0)
            mask_b = wp.tile([128, N], F32)
            mb_src = p_mask[0:1]
            nc.sync.dma_start(
                out=mask_b,
                in_=bass.AP(tensor=mb_src.tensor, offset=0, ap=[[0, 128], [1, N]]),
            )

            # fp8 DR weight tile for layer1: cols 0-511 = 64*[W1r; AF'],
            # cols 512*(r+1).. = 64*NA[r]
            wdr = wp.tile([128, 2048], F8)
            nc.vector.tensor_scalar_mul(wdr[0:64, 0:512], w1r, FS)
            # fp8 DR weights for layer2 (64*W1b, col = k*512+f layout)
            w1b_dr = wp.tile([128, 2048], F8)
            nc.vector.tensor_scalar_mul(w1b_dr, w1b, FS)

            # ================= phase 0: geometry =================
            axs = wp.tile([64, 3], F32)
            nc.vector.tensor_tensor(axs, ax_sb, ax_sb, op=mybir.AluOpType.mult)
            aa2 = wp.tile([64, 1], F32)
            nc.vector.reduce_sum(aa2, axs, axis=mybir.AxisListType.X)
            aug_a = wp.tile([64, 5], F32)
            nc.vector.tensor_scalar_mul(aug_a[:, 0:3], ax_sb, -2.0)
            nc.vector.memset(aug_a[:, 3:4], 1.0)
            nc.vector.tensor_copy(aug_a[:, 4:5], aa2)
            p_t = psA.tile([128, 64], F32, tag="a")
            nc.tensor.transpose(p_t[0:5, 0:64], aug_a, ident[0:64, 0:64])
            axaug = wp.tile([5, 64], F32)
            nc.vector.tensor_copy(axaug, p_t[0:5, 0:64])


            # bf16 weights for the NA / AF' precomputations
            w1nf_b = wp.tile([128, 1024], BF16); nc.vector.tensor_copy(w1nf_b, w1nf)
            w1af_b = wp.tile([128, 1024], BF16); nc.vector.tensor_copy(w1af_b, w1af)

            nfT_b = wp.tile([128, 768], BF16)   # nf.T bf16, kc-major
            nfT_f = wp.tile([128, 768], F32)    # nf.T f32
            # --- batched distance pipeline: all 3 node blocks in one pass ---
            aug_n = wp.tile([128, 15], F32)
            nxs = wp.tile([128, 3], F32)
            for r in range(3):
                nc.vector.tensor_tensor(nxs, nx_sb[r], nx_sb[r], op=mybir.AluOpType.mult)
                nc.vector.reduce_sum(aug_n[:, 5 * r + 3:5 * r + 4], nxs,
                                     axis=mybir.AxisListType.X)
                nc.vector.tensor_copy(aug_n[:, 5 * r:5 * r + 3], nx_sb[r])
                nc.vector.memset(aug_n[:, 5 * r + 4:5 * r + 5], 1.0)
            nxaugT = wp.tile([128, 128], F32)
            axaug3 = wp.tile([128, 64], F32)
            for r in range(3):
                p_tn = psB.tile([128, 128], F32, tag="b", name=f"ptn{r}")
                nc.tensor.transpose(p_tn[0:5, :], aug_n[:, 5 * r:5 * r + 5], ident)
                nc.vector.tensor_copy(nxaugT[32 * r:32 * r + 5, :], p_tn[0:5, :])
                nc.vector.tensor_copy(axaug3[32 * r:32 * r + 5, :], axaug)
            p_d2 = psB.tile([128, 512], F32, tag="b")
            for r in range(3):
                nc.tensor.matmul(p_d2[:, 64 * r:64 * r + 64],
                                 nxaugT[32 * r:32 * r + 5, :],
                                 axaug3[32 * r:32 * r + 5, :],
                                 start=True, stop=True)
            d2c = wp.tile([128, 192], F32)
            nc.vector.tensor_scalar_max(d2c, p_d2[:, 0:192], 0.0)
            t_sb = wp.tile([128, 192], F32)
            nc.scalar.activation(t_sb, d2c, AF.Sqrt, bias=zcol[:, 0:1], scale=D2_SCALE)
            nc.gpsimd.dma_start(t_dram3[:, :], t_sb)

            for r in range(3):
                # nf transpose blocks + NA matmuls
                for c in range(2):
                    p_tr = psB.tile([128, 128], F32, tag="b")
                    nc.tensor.transpose(p_tr, nf_sb[r][:, 128 * c:128 * c + 128], ident)
                    nc.vector.tensor_copy(
                        nfT_b[:, 384 * c + 128 * r:384 * c + 128 * r + 128], p_tr)
                    nc.vector.tensor_copy(
                        nfT_f[:, 384 * c + 128 * r:384 * c + 128 * r + 128], p_tr)
            for r in range(3):
                p_na = psB.tile([128, 512], F32, tag="b")
                for kc in range(2):
                    nc.tensor.matmul(
                        p_na,
                        nfT_b[:, 384 * kc + 128 * r:384 * kc + 128 * r + 128],
                        w1nf_b[:, 512 * kc:512 * kc + 512],
                        start=(kc == 0), stop=(kc == 1),
                    )
                nc.vector.tensor_scalar_mul(
                    wdr[:, 512 * (r + 1):512 * (r + 2)], p_na, FS)

            # af.T and AF' = af@W1a[H:2H] + b1a -> wdr rows 64-127
            afT_b = wp.tile([128, 128], BF16)
            for c in range(2):
                p_tr = psA.tile([128, 64], F32, tag="a")
                nc.tensor.transpose(p_tr[:, 0:64], af_sb[:, 128 * c:128 * c + 128],
                                    ident[0:64, 0:64])
                nc.vector.tensor_copy(afT_b[:, 64 * c:64 * c + 64], p_tr[:, 0:64])
            p_af2 = psB.tile([64, 512], F32, tag="b")
            for kc in range(2):
                nc.tensor.matmul(p_af2, afT_b[:, 64 * kc:64 * kc + 64],
                                 w1af_b[:, 512 * kc:512 * kc + 512],
                                 start=(kc == 0), stop=False)
            nc.tensor.matmul(p_af2, ones64, b1a_rb, start=False, stop=True)
            nc.vector.tensor_scalar_mul(wdr[64:128, 0:512], p_af2, FS)

            sT = [wp.tile([128, 512], BF16, tag=f"sT{r}", name=f"sT{r}")
                  for r in range(3)]  # per-node-block anchor-sums (fc-major x node)

            def ap3(t2d, kstride, kn, inner):
                """[p, kn, inner] AP from a 2D slice (middle dim stride kstride)."""
                return bass.AP(tensor=t2d.tensor, offset=t2d.offset,
                               ap=[list(t2d.ap[0]), [kstride, kn], [1, inner]])

            # ================= main loop (software-pipelined) =================
            def stageA(pp):
                """t gather + rbf for superchunk pair (2pp, 2pp+1)."""
                tb = tbp.tile([128, 1024], F32, tag="tb", name=f"tb{pp}")
                for h in range(2):
                    sc = 2 * pp + h
                    nc.sync.dma_start(
                        out=tb[64 * h:64 * h + 64, :].rearrange(
                            "p (n a) -> p n a", a=64),
                        in_=bass.AP(tensor=t_dram3[:].tensor,
                                    offset=(16 * sc % 128) * 192 + (sc // 8) * 64,
                                    ap=[[0, 64], [192, 16], [1, 64]]),
                    )
                qx = qp.tile([128, 1024], F32, tag="qx", name=f"qx{pp}")
                nc.scalar.activation(qx, tb, AF.Square, bias=negmu[:, 0:1], scale=1.0)
                for h in range(2):
                    c0 = (2 * pp + h) * 2
                    v0 = 8 * ((c0 // 8) % 2) + c0 % 8
                    ov = vt[v0]
                    out_ap = bass.AP(tensor=ov.tensor, offset=ov.offset,
                                     ap=[[ov.ap[0][0], 64], [1024, 2], [1, 512]])
                    nc.scalar.activation(
                        out_ap, qx[64 * h:64 * h + 64, :].rearrange(
                            "p (t x) -> p t x", t=2),
                        AF.Exp, bias=zcol[64 * h:64 * h + 64, 0:1], scale=-1.0)

            def stageB(c):
                """L1 fp8-DR matmuls + relu -> h1 (fp8)."""
                r = c // 16
                v = 8 * ((c // 8) % 2) + c % 8
                rhs = ap3(vt[v][:, 0:512], 512, 2, 512)
                h1 = h1p.tile([128, 2048], F8, tag="h1", name=f"h1_{c}")
                for hh in range(2):
                    p1 = psA.tile([128, 1024], F32, tag="a", name=f"p1_{c}_{hh}")
                    for i in range(2):
                        fc = 2 * hh + i
                        lhsT = ap3(wdr[:, 128 * fc:128 * fc + 128],
                                   512 * (r + 1), 2, 128)
                        nc.tensor.matmul(p1[:, 512 * i:512 * i + 512], lhsT, rhs,
                                         start=True, stop=True, perf_mode=DR)
                    if hh == 0:
                        nc.scalar.activation(h1[:, 0:1024], p1, AF.Relu,
                                             bias=zcol[:, 0:1], scale=1.0 / FS)
                    else:
                        zap = bass.AP(tensor=zcol.tensor, offset=zcol.offset,
                                      ap=[list(zcol.ap[0]), [0, 1024]])
                        with nc.allow_low_precision(reason="fp8 h1 feeds fp8 matmul"):
                            nc.vector.scalar_tensor_tensor(
                                h1[:, 1024:2048], p1, 1.0 / FS, zap,
                                op0=mybir.AluOpType.mult, op1=mybir.AluOpType.max)
                return h1

            h2_live = {}

            def stageC1(c, h1):
                """L2 fp8-DR matmuls + relu (ACT/DVE split) + Pool L1 halving."""
                h2t = h2p.tile([128, 2048], BF16, tag="h2", name=f"h2_{c}")
                for fc in range(4):
                    p2 = psB.tile([128, 512], F32, tag="b", name=f"p2_{c}_{fc}")
                    for kc in range(2):
                        lhsT = ap3(
                            w1b_dr[:, 1024 * kc + 128 * fc:1024 * kc + 128 * fc + 128],
                            512, 2, 128)
                        rhs = ap3(h1[:, 1024 * kc:1024 * kc + 512], 512, 2, 512)
                        nc.tensor.matmul(p2, lhsT, rhs,
                                         start=(kc == 0), stop=(kc == 1),
                                         perf_mode=DR)
                    # h2 kept 64x-scaled (scale folded into layer-3):
                    # ACT for fc<2, DVE (max(z,-64b)+64b == relu(z+64b)) else
                    if fc < 2:
                        nc.scalar.activation(h2t[:, 512 * fc:512 * fc + 512],
                                             p2, AF.Relu,
                                             bias=b1b64_c[:, fc:fc + 1],
                                             scale=1.0)
                    else:
                        bcol = b1b64_c[:, fc:fc + 1]
                        bbc = bass.AP(tensor=bcol.tensor, offset=bcol.offset,
                                      ap=[list(bcol.ap[0]), [0, 512]])
                        with nc.allow_low_precision(reason="bf16 h2 partial sums"):
                            nc.vector.scalar_tensor_tensor(
                                h2t[:, 512 * fc:512 * fc + 512], p2,
                                nb1b64_c[:, fc:fc + 1], bbc,
                                op0=mybir.AluOpType.max,
                                op1=mybir.AluOpType.add)
                # anchor-sum level 1 on Pool (single batched instr)
                rha = redp.tile([128, 1024], BF16, tag="rha", name=f"rha_{c}")
                hv = h2t.rearrange("p (x t a) -> p x t a", t=2, a=32)
                with nc.allow_low_precision(reason="bf16 h2 partial sums"):
                    nc.gpsimd.tensor_tensor(
                        rha.rearrange("p (x a) -> p x a", a=32),
                        hv[:, :, 0, :], hv[:, :, 1, :], op=mybir.AluOpType.add)
                return rha

            def stageC2(c, rha):
                """Anchor-sum: Pool L2 halving + DVE segmented reduce into sT."""
                r = c // 16
                rhb = redp.tile([128, 512], BF16, tag="rhb", name=f"rhb_{c}")
                rv = rha.rearrange("p (x t a) -> p x t a", t=2, a=16)
                with nc.allow_low_precision(reason="bf16 h2 partial sums"):
                    nc.gpsimd.tensor_tensor(
                        rhb.rearrange("p (x a) -> p x a", a=16),
                        rv[:, :, 0, :], rv[:, :, 1, :], op=mybir.AluOpType.add)
                with nc.allow_low_precision(reason="DVE reduce accumulates fp32 internally; bf16 sT feeds bf16 matmul"):
                    nc.vector.tensor_reduce(
                        sT[r].rearrange("p (f n) -> p f n", n=128)[
                            :, 0:4, 8 * (c % 16):8 * (c % 16) + 8],
                        rhb.rearrange("p (f n a) -> p f n a", n=8, a=16),
                        axis=mybir.AxisListType.X,
                        op=mybir.AluOpType.add,
                    )

            # ====== phase 2 (node path), per-128-node block, overlapped ======
            zeros_b = wp.tile([128, 128], BF16)
            nc.vector.memset(zeros_b, 0.0)
            zeros_f = wp.tile([128, 128], F32)
            nc.vector.memset(zeros_f, 0.0)
            p2w = {}

            def load_phase2_weights():
                w1c = wp.tile([128, 1024], F32, name="w1c", tag="w1c")
                nc.scalar.dma_start(w1c.rearrange("p (k f) -> p k f", k=4), p_w1c[:].rearrange("(k p) f -> p k f", p=128))
                w2a = wp.tile([128, 1024], F32, name="w2a", tag="w2a")
                nc.scalar.dma_start(w2a.rearrange("p (k f) -> p k f", k=2), p_w2a[:].rearrange("(k p) f -> p k f", p=128))
                w2b = wp.tile([128, 2048], F32, name="w2b", tag="w2b")
                nc.scalar.dma_start(w2b.rearrange("p (k f) -> p k f", k=4), p_w2b[:].rearrange("(k p) f -> p k f", p=128))
                w2c = wp.tile([128, 1024], F32, name="w2c", tag="w2c")
                nc.scalar.dma_start(w2c.rearrange("p (k f) -> p k f", k=4), p_w2c[:].rearrange("(k p) f -> p k f", p=128))
                for nm, t in (("w1c", w1c), ("w2a", w2a), ("w2b", w2b), ("w2c", w2c)):
                    b = wp.tile(list(t.shape), BF16, name=nm + "_b", tag=nm + "_b")
                    nc.gpsimd.tensor_copy(b, t)
                    p2w[nm] = b

            def block_ln(r, x_in, g_c, b_c, out_t):
                """LN over 256 feats for 128 nodes; x_in/out_t [128, 256] fo-major.
                Cross-partition sums via all-ones fp32 PE matmuls (broadcast to
                all partitions); rstd = exp(-0.5*ln(v)) keeps one ACT table."""
                x3 = x_in.rearrange("p (f n) -> p f n", f=2)
                red = psC.tile([128, 128], F32, tag="c", name=f"lnr{r}")
                nc.tensor.matmul(red, ones_f, x_in[:, 0:128], start=True, stop=False)
                nc.tensor.matmul(red, ones_f, x_in[:, 128:256], start=False, stop=True)
                Sb = bass.AP(tensor=red.tensor, offset=red.offset,
                             ap=[list(red.ap[0]), [0, 2], list(red.ap[1])])
                xc = wp.tile([128, 256], F32, tag=f"lnxc{r}", name=f"lnxc{r}")
                nc.vector.scalar_tensor_tensor(
                    xc.rearrange("p (f n) -> p f n", f=2), Sb, -1.0 / 256.0, x3,
                    op0=mybir.AluOpType.mult, op1=mybir.AluOpType.add)
                sq = wp.tile([128, 256], F32, tag=f"lnsq{r}", name=f"lnsq{r}")
                nc.vector.tensor_tensor(sq, xc, xc, op=mybir.AluOpType.mult)
                red2 = psC.tile([128, 128], F32, tag="c", name=f"lnr2{r}")
                nc.tensor.matmul(red2, ones_f, sq[:, 0:128], start=True, stop=False)
                nc.tensor.matmul(red2, ones_f, sq[:, 128:256], start=False, stop=True)
                lnv = wp.tile([128, 128], F32, tag=f"lnv{r}", name=f"lnv{r}")
                nc.scalar.activation(lnv, red2, AF.Ln, bias=eps_c[:, 0:1],
                                     scale=1.0 / 256.0)
                rstd = wp.tile([128, 128], F32, tag=f"lnrstd{r}", name=f"lnrstd{r}")
                nc.scalar.activation(rstd, lnv, AF.Exp, bias=zcol[:, 0:1], scale=-0.5)
                rb = bass.AP(tensor=rstd.tensor, offset=rstd.offset,
                             ap=[list(rstd.ap[0]), [0, 2], list(rstd.ap[1])])
                y = wp.tile([128, 256], F32, tag=f"lny{r}", name=f"lny{r}")
                nc.vector.tensor_tensor(y.rearrange("p (f n) -> p f n", f=2),
                                        xc.rearrange("p (f n) -> p f n", f=2), rb,
                                        op=mybir.AluOpType.mult)
                for fo in range(2):
                    bcc = b_c[:, fo:fo + 1]
                    bccb = bass.AP(tensor=bcc.tensor, offset=bcc.offset,
                                   ap=[list(bcc.ap[0]), [0, 128]])
                    nc.vector.scalar_tensor_tensor(
                        out_t[:, 128 * fo:128 * fo + 128],
                        y[:, 128 * fo:128 * fo + 128], g_c[:, fo:fo + 1], bccb,
                        op0=mybir.AluOpType.mult, op1=mybir.AluOpType.add)

            p2state = {}

            def phase2_piece(r, piece):
                w1c_b, w2a_b = p2w["w1c"], p2w["w2a"]
                w2b_b, w2c_b = p2w["w2b"], p2w["w2c"]
                psP = psB if r == 2 else psC
                ptag = "b" if r == 2 else "c"
                st = p2state.setdefault(r, {})
                if piece == 0:
                    # L3 + x1 + LN1 (+ bf16 cast)
                    upd = wp.tile([128, 256], F32, tag=f"upd{r}", name=f"upd{r}")
                    for fo in range(2):
                        p3 = psP.tile([128, 128], F32, tag=ptag, name=f"p3_{r}_{fo}")
                        for kc in range(4):
                            nc.tensor.matmul(
                                p3,
                                w1c_b[:, 256 * kc + 128 * fo:256 * kc + 128 * fo + 128],
                                sT[r][:, 128 * kc:128 * kc + 128],
                                start=(kc == 0), stop=(kc == 3),
                            )
                        bcc = b1c_c[:, fo:fo + 1]
                        bccb = bass.AP(tensor=bcc.tensor, offset=bcc.offset,
                                       ap=[list(bcc.ap[0]), [0, 128]])
                        nc.vector.scalar_tensor_tensor(
                            upd[:, 128 * fo:128 * fo + 128], p3,
                            1.0 / (64.0 * 64.0), bccb,
                            op0=mybir.AluOpType.mult, op1=mybir.AluOpType.add)
                    x1 = wp.tile([128, 256], F32, tag=f"x1{r}", name=f"x1{r}")
                    tmpw = wp.tile([128, 256], F32, tag=f"updm{r}", name=f"updm{r}")
                    mb = mask_b[:, 128 * r:128 * r + 128]
                    mb2 = bass.AP(tensor=mb.tensor, offset=mb.offset,
                                  ap=[list(mb.ap[0]), [0, 2], list(mb.ap[1])])
                    nc.vector.tensor_tensor(
                        tmpw.rearrange("p (f n) -> p f n", f=2),
                        upd.rearrange("p (f n) -> p f n", f=2), mb2,
                        op=mybir.AluOpType.mult)
                    nfs = nfT_f[:, 128 * r:128 * r + 128]
                    nfv = bass.AP(tensor=nfs.tensor, offset=nfs.offset,
                                  ap=[list(nfs.ap[0]), [384, 2], list(nfs.ap[1])])
                    nc.vector.tensor_tensor(
                        x1.rearrange("p (f n) -> p f n", f=2),
                        tmpw.rearrange("p (f n) -> p f n", f=2), nfv,
                        op=mybir.AluOpType.add)
                    nf1 = wp.tile([128, 256], F32, tag=f"nf1{r}", name=f"nf1{r}")
                    block_ln(r, x1, ln1g_c, ln1b_c, nf1)
                    nf1_b = wp.tile([128, 256], BF16, tag=f"nf1b{r}", name=f"nf1b{r}")
                    nc.vector.tensor_copy(nf1_b, nf1)
                    st["nf1"], st["nf1_b"] = nf1, nf1_b
                elif piece == 1:
                    nf1_b = st["nf1_b"]
                    g1 = wp.tile([128, 512], BF16, tag=f"g1{r}", name=f"g1{r}")
                    for fc in range(4):
                        p = psP.tile([128, 128], F32, tag=ptag, name=f"pg1_{r}_{fc}")
                        for kc in range(2):
                            nc.tensor.matmul(
                                p, w2a_b[:, 512 * kc + 128 * fc:512 * kc + 128 * fc + 128],
                                nf1_b[:, 128 * kc:128 * kc + 128],
                                start=(kc == 0), stop=(kc == 1))
                        nc.vector.scalar_tensor_tensor(
                            g1[:, 128 * fc:128 * fc + 128], p, b2a_c[:, fc:fc + 1],
                            zeros_b, op0=mybir.AluOpType.add, op1=mybir.AluOpType.max)
                    st["g1"] = g1
                elif piece == 2:
                    g1 = st["g1"]
                    g2 = wp.tile([128, 512], BF16, tag=f"g2{r}", name=f"g2{r}")
                    for fc in range(4):
                        p = psP.tile([128, 128], F32, tag=ptag, name=f"pg2_{r}_{fc}")
                        for kc in range(4):
                            nc.tensor.matmul(
                                p, w2b_b[:, 512 * kc + 128 * fc:512 * kc + 128 * fc + 128],
                                g1[:, 128 * kc:128 * kc + 128],
                                start=(kc == 0), stop=(kc == 3))
                        nc.vector.scalar_tensor_tensor(
                            g2[:, 128 * fc:128 * fc + 128], p, b2b_c[:, fc:fc + 1],
                            zeros_b, op0=mybir.AluOpType.add, op1=mybir.AluOpType.max)
                    st["g2"] = g2
                elif piece == 3:
                    g2, nf1 = st["g2"], st["nf1"]
                    upd2 = wp.tile([128, 256], F32, tag=f"upd2{r}", name=f"upd2{r}")
                    for fo in range(2):
                        p = psP.tile([128, 128], F32, tag=ptag, name=f"pu2_{r}_{fo}")
                        for kc in range(4):
                            nc.tensor.matmul(
                                p, w2c_b[:, 256 * kc + 128 * fo:256 * kc + 128 * fo + 128],
                                g2[:, 128 * kc:128 * kc + 128],
                                start=(kc == 0), stop=(kc == 3))
                        nc.vector.scalar_tensor_tensor(
                            upd2[:, 128 * fo:128 * fo + 128], p, b2c_c[:, fo:fo + 1],
                            zeros_f, op0=mybir.AluOpType.add, op1=mybir.AluOpType.add)
                    x2 = wp.tile([128, 256], F32, tag=f"x2{r}", name=f"x2{r}")
                    tmpw2 = wp.tile([128, 256], F32, tag=f"updm2{r}", name=f"updm2{r}")
                    mb = mask_b[:, 128 * r:128 * r + 128]
                    mb2 = bass.AP(tensor=mb.tensor, offset=mb.offset,
                                  ap=[list(mb.ap[0]), [0, 2], list(mb.ap[1])])
                    nc.vector.tensor_tensor(
                        tmpw2.rearrange("p (f n) -> p f n", f=2),
                        upd2.rearrange("p (f n) -> p f n", f=2), mb2,
                        op=mybir.AluOpType.mult)
                    nc.vector.tensor_tensor(x2, tmpw2, nf1, op=mybir.AluOpType.add)
                    outT = wp.tile([128, 256], F32, tag=f"outT{r}", name=f"outT{r}")
                    block_ln(r, x2, ln2g_c, ln2b_c, outT)
                    st["outT"] = outT
                else:
                    outT = st["outT"]
                    o_sb = outp.tile([128, 256], F32, tag="osb", name=f"osb{r}")
                    for fo in range(2):
                        p_tr = psP.tile([128, 128], F32, tag=ptag, name=f"ptr_{r}_{fo}")
                        nc.tensor.transpose(p_tr, outT[:, 128 * fo:128 * fo + 128], ident)
                        nc.vector.tensor_copy(o_sb[:, 128 * fo:128 * fo + 128], p_tr)
                    nc.sync.dma_start(p_out[128 * r:128 * r + 128, :], o_sb)

            for _pp in range(2):
                stageA(_pp)
            h1_live = {}
            for c in range(NC_CHUNKS + 5):
                if c == 8:
                    load_phase2_weights()
                if 0 <= c - 4 < NC_CHUNKS:
                    stageC2(c - 4, h2_live.pop(c - 4))
                if 0 <= c - 3 < NC_CHUNKS:
                    h2_live[c - 3] = stageC1(c - 3, h1_live.pop(c - 3))
                if c % 4 == 0 and c // 4 + 2 < 12:
                    stageA(c // 4 + 2)
                if c < NC_CHUNKS:
                    h1_live[c] = stageB(c)
                for rr, base in ((0, 21), (1, 37)):
                    if base <= c < base + 10 and (c - base) % 2 == 0:
                        phase2_piece(rr, (c - base) // 2)
                if c == 52:
                    for pp in range(5):
                        phase2_piece(2, pp)

    nc.compile()
    return nc


_NC = None
_CONSTS = _consts()


def build_in_maps(inputs):
    shared = {k: np.ascontiguousarray(np.asarray(inputs[k], np.float32))
              for k in ("W1a", "b1a", "W1b", "b1b", "W1c", "b1c", "ln1_g", "ln1_b",
                        "W2a", "b2a", "W2b", "b2b", "W2c", "b2c", "ln2_g", "ln2_b")}
    shared.update(_CONSTS)
    shared["c_biases"] = _pack_biases(inputs)

    node_x = np.asarray(inputs["node_x"], np.float32)
    anchor_x = np.asarray(inputs["anchor_x"], np.float32)
    node_features = np.asarray(inputs["node_features"], np.float32)
    anchor_features = np.asarray(inputs["anchor_features"], np.float32)
    node_mask = np.asarray(inputs["node_mask"], np.float32)

    in_maps = []
    for b in range(B):
        m = dict(shared)
        m["node_x"] = np.ascontiguousarray(node_x[b * N:(b + 1) * N])
        m["anchor_x"] = np.ascontiguousarray(anchor_x[b * A:(b + 1) * A])
        m["node_features"] = np.ascontiguousarray(node_features[b * N:(b + 1) * N])
        m["anchor_features"] = np.ascontiguousarray(
            anchor_features[b * A:(b + 1) * A])
        m["node_mask"] = np.ascontiguousarray(node_mask[b * N:(b + 1) * N])
        in_maps.append(m)
    return in_maps


def kernel(**inputs):
    global _NC
    if _NC is None:
        _NC = _build()
    in_maps = build_in_maps(inputs)
    res = run_bass_kernel_spmd(_NC, in_maps, core_ids=list(range(B)))
    return np.concatenate([res.results[b]["out"] for b in range(B)], axis=0)


# revision 50
# speedup vs baseline: 1.2046x; 1.2046x over previous
"""AnchorSet2NodeMPNN Trainium2 kernel (8 NeuronCores, graph-parallel).

Each core handles one graph (N=384 nodes, A=64 anchors, H=256, E=64).

Algorithmic structure (per core):
  d^2[n,a] = |nx|^2 + |ax|^2 - 2 nx.ax           (rank-5 K=5 matmul)
  t'[n,a]  = sqrt(d^2) * (0.1/sigma)              (ACT sqrt; staged to DRAM)
  Layer 1 is factored: pair @ W1a = nf@W1a[:H] + af@W1a[H:2H] + rbf@W1a[2H:]
    - NA = nf@W1a[:H]    per-node   (computed once)
    - AF' = af@W1a[H:2H] + b1a  per-anchor (computed once)
    All three terms are fused into ONE fp8 DoubleRow matmul with K=256:
      ko=0: rows 0-63 = 64*W1r (rbf weights), rows 64-127 = 64*AF' with
            anchor-indicator rhs; ko=1: 64*NA with node-indicator rhs.
    Per chunk (8 nodes x 64 anchors) the rhs is a per-variant static tile
    [rbf | mast_a | mast_n] where only the rbf quarter is rewritten.
  Layer 2 per-pair dense in fp8 DoubleRow (K=512 as 2 passes of 256),
    weights scaled x64 to stay in fp8-normal range; relu rescales by 1/64.
  Layer 3 commutes with the anchor-mean: upd = (sum_a h2) @ W1c / 64^2 + b1c
    (h2 is kept 64x-scaled so relu biases stay per-partition on ACT/DVE).
  Anchor-sum: two halving adds on GPSIMD (Pool) + short DVE tensor_reduce.
  LayerNorms run feature-transposed; cross-partition sums use an all-ones
  fp32 PE matmul (result broadcast to every partition); 1/sqrt(v) is
  exp(-0.5*ln(v)) (no DVE-reciprocal roundtrip).

Engine balance per chunk: PE 12 DR matmuls; ACT relu-h1(half)+relu-h2(2/4)
+rbf square/exp; DVE relu-h1(half)+relu-h2(2/4)+final reduce; Pool halvings.
Activation float-biases are passed as a preloaded zero column, never 0.0
immediates (a float bias materializes a const AP whose DMA would serialize
the ACT queue behind the weight-load flood).

fp8 quantization error (vs fp32 reference) is ~2.2e-3 overall; anchor-mean
averaging suppresses activation-quantization noise.
"""
import numpy as np
import ml_dtypes

import concourse.bass as bass
import concourse.mybir as mybir
import concourse.tile as tile
from concourse import bacc
from concourse.bass_utils import run_bass_kernel_spmd

F32 = mybir.dt.float32
BF16 = mybir.dt.bfloat16
F8 = mybir.dt.float8e4
AF = mybir.ActivationFunctionType
DR = mybir.MatmulPerfMode.DoubleRow

B, N, A, H, E = 8, 384, 64, 256, 64
RBF_D_MAX = 20.0
SIGMA = RBF_D_MAX / E                    # 0.3125
MU = np.linspace(0.0, RBF_D_MAX, E).astype(np.float32)
NC_CHUNKS = 48                           # sub-chunks of 8 nodes x 64 anchors
D2_SCALE = 0.01 / SIGMA**2               # t' = sqrt(d2 * D2_SCALE) = (d/10)/sigma
FS = 64.0                                # fp8 weight scale


def _consts():
    # mast_a[a', j] = 1 if a' == j % 64
    mast_a = np.zeros((64, 512), np.float32)
    for j in range(512):
        mast_a[j % 64, j] = 1.0
    # mast_n[k, 512*(8q+s) + j] = 1 if k == 64q + 8s + j//64
    mast_n = np.zeros((128, 8192), np.float32)
    for q in range(2):
        for s in range(8):
            for j in range(512):
                k = 64 * q + 8 * s + j // 64
                mast_n[k, 4096 * q + 512 * s + j] = 1.0
    ident = np.eye(128, dtype=np.float32)
    negmusig = np.tile(-(MU / SIGMA).astype(np.float32), 2)
    ones64 = np.ones((1, 64), np.float32)
    return dict(
        c_mast_a=mast_a.astype(ml_dtypes.float8_e4m3),
        c_mast_n=mast_n.astype(ml_dtypes.float8_e4m3),
        c_ident=ident,
        c_negmusig=negmusig,
        c_ones64=ones64.astype(ml_dtypes.bfloat16),
    )


def _pack_biases(inputs):
    """Column-pack per-feature vectors: [128, 24] f32, layout-only."""
    cols = []
    for k, n in (("b1b", 4), ("b1c", 2), ("b2a", 4), ("b2b", 4), ("b2c", 2),
                 ("ln1_g", 2), ("ln1_b", 2), ("ln2_g", 2), ("ln2_b", 2)):
        v = np.asarray(inputs[k], np.float32)
        cols.append(v.reshape(n, 128).T)
    b64 = 64.0 * np.asarray(inputs["b1b"], np.float32)
    cols.append(b64.reshape(4, 128).T)
    cols.append(-b64.reshape(4, 128).T)
    return np.ascontiguousarray(np.concatenate(cols, axis=1))


def _build():
    nc = bacc.Bacc("TRN2", target_bir_lowering=False, debug=False)

    # ---- parameters ----
    p_nx = nc.declare_dram_parameter("node_x", [N, 3], F32, isOutput=False)
    p_ax = nc.declare_dram_parameter("anchor_x", [A, 3], F32, isOutput=False)
    p_nf = nc.declare_dram_parameter("node_features", [N, H], F32, isOutput=False)
    p_af = nc.declare_dram_parameter("anchor_features", [A, H], F32, isOutput=False)
    p_mask = nc.declare_dram_parameter("node_mask", [N], F32, isOutput=False)
    p_w1a = nc.declare_dram_parameter("W1a", [2 * H + E, 512], F32, isOutput=False)
    p_b1a = nc.declare_dram_parameter("b1a", [512], F32, isOutput=False)
    p_w1b = nc.declare_dram_parameter("W1b", [512, 512], F32, isOutput=False)
    p_b1b = nc.declare_dram_parameter("b1b", [512], F32, isOutput=False)
    p_w1c = nc.declare_dram_parameter("W1c", [512, H], F32, isOutput=False)
    p_b1c = nc.declare_dram_parameter("b1c", [H], F32, isOutput=False)
    p_ln1g = nc.declare_dram_parameter("ln1_g", [H], F32, isOutput=False)
    p_ln1b = nc.declare_dram_parameter("ln1_b", [H], F32, isOutput=False)
    p_w2a = nc.declare_dram_parameter("W2a", [H, 512], F32, isOutput=False)
    p_b2a = nc.declare_dram_parameter("b2a", [512], F32, isOutput=False)
    p_w2b = nc.declare_dram_parameter("W2b", [512, 512], F32, isOutput=False)
    p_b2b = nc.declare_dram_parameter("b2b", [512], F32, isOutput=False)
    p_w2c = nc.declare_dram_parameter("W2c", [512, H], F32, isOutput=False)
    p_b2c = nc.declare_dram_parameter("b2c", [H], F32, isOutput=False)
    p_ln2g = nc.declare_dram_parameter("ln2_g", [H], F32, isOutput=False)
    p_ln2b = nc.declare_dram_parameter("ln2_b", [H], F32, isOutput=False)
    c_mast_a = nc.declare_dram_parameter("c_mast_a", [64, 512], F8, isOutput=False)
    c_mast_n = nc.declare_dram_parameter("c_mast_n", [128, 8192], F8, isOutput=False)
    c_biases = nc.declare_dram_parameter("c_biases", [128, 32], F32, isOutput=False)
    c_ident = nc.declare_dram_parameter("c_ident", [128, 128], F32, isOutput=False)
    c_negmusig = nc.declare_dram_parameter("c_negmusig", [2 * E], F32, isOutput=False)
    c_ones64 = nc.declare_dram_parameter("c_ones64", [1, 64], BF16, isOutput=False)
    p_out = nc.declare_dram_parameter("out", [N, H], F32, isOutput=True)

    t_dram3 = nc.dram_tensor("t_scratch", [128, 3 * A], F32)

    with tile.TileContext(nc) as tc:
        with (
            tc.tile_pool(name="wp", bufs=1) as wp,
            tc.tile_pool(name="psA", bufs=2, space="PSUM") as psA,
            tc.tile_pool(name="psB", bufs=3, space="PSUM") as psB,
            tc.tile_pool(name="psC", bufs=1, space="PSUM") as psC,
            tc.tile_pool(name="tbp", bufs=3) as tbp,
            tc.tile_pool(name="qp", bufs=2) as qp,
            tc.tile_pool(name="h1p", bufs=4) as h1p,
            tc.tile_pool(name="h2p", bufs=6) as h2p,
            tc.tile_pool(name="redp", bufs=4) as redp,
            tc.tile_pool(name="outp", bufs=2) as outp,
        ):
            dma = nc.sync.dma_start

            # ================= phase 0: loads =================
            nx_sb = [wp.tile([128, 3], F32, tag=f"nx{r}", name=f"nx{r}")
                     for r in range(3)]
            for r in range(3):
                dma(nx_sb[r], p_nx[128 * r:128 * r + 128, :])
            ax_sb = wp.tile([64, 3], F32)
            dma(ax_sb, p_ax[:])
            ident = wp.tile([128, 128], F32); dma(ident, c_ident[:])
            negmu = wp.tile([128, 1], F32)
            dma(negmu, c_negmusig[:].rearrange("(p o) -> p o", o=1))
            w1nf = wp.tile([128, 1024], F32)
            nc.gpsimd.dma_start(w1nf.rearrange("p (k f) -> p k f", k=2), p_w1a[0:256, :].rearrange("(k p) f -> p k f", p=128))
            w1af = wp.tile([128, 1024], F32)
            nc.gpsimd.dma_start(w1af.rearrange("p (k f) -> p k f", k=2), p_w1a[256:512, :].rearrange("(k p) f -> p k f", p=128))
            w1r = wp.tile([64, 512], F32)
            dma(w1r, p_w1a[512:576, :])
            nf_sb = [wp.tile([128, 256], F32, tag=f"nfsb{r}", name=f"nfsb{r}")
                     for r in range(3)]
            for r in range(3):
                nc.gpsimd.dma_start(nf_sb[r], p_nf[128 * r:128 * r + 128, :])
            # 16 variant blocks in one parent tile, each 1024 cols:
            # [rbf (dynamic) | mast_a rows 64-127 | mast_n variant]
            vtp = wp.tile([128, 16384], F8)
            vt = [vtp[:, 1024 * v:1024 * v + 1024] for v in range(16)]
            nc.gpsimd.dma_start(
                out=vtp[64:128, :].rearrange("p (v x) -> p v x", x=1024)[:, :, 0:512],
                in_=bass.AP(tensor=c_mast_a[:].tensor, offset=0,
                            ap=[[512, 64], [0, 16], [1, 512]]),
            )
            nc.gpsimd.dma_start(
                out=vtp.rearrange("p (v x) -> p v x", x=1024)[:, :, 512:1024],
                in_=c_mast_n[:].rearrange("p (v x) -> p v x", x=512),
            )
            b1a_row = wp.tile([1, 512], F32)
            dma(b1a_row, p_b1a[:].rearrange("(o f) -> o f", o=1))
            ones64 = wp.tile([1, 64], BF16); dma(ones64, c_ones64[:])
            af_sb = wp.tile([64, 256], F32)
            nc.gpsimd.dma_start(af_sb, p_af[:])
            w1b = wp.tile([128, 2048], F32)
            nc.gpsimd.dma_start(w1b.rearrange("p (k f) -> p k f", k=4), p_w1b[:].rearrange("(k p) f -> p k f", p=128))

            # packed bias columns
            bias_pack = wp.tile([128, 32], F32)
            dma(bias_pack, c_biases[:])
            off = [0]
            def bp(n):
                t = bias_pack[:, off[0]:off[0] + n]
                off[0] += n
                return t
            b1b_c = bp(4); b1c_c = bp(2); b2a_c = bp(4); b2b_c = bp(4)
            b2c_c = bp(2); ln1g_c = bp(2); ln1b_c = bp(2); ln2g_c = bp(2)
            ln2b_c = bp(2); b1b64_c = bp(4); nb1b64_c = bp(4)
            b1a_rb = wp.tile([1, 512], BF16); nc.vector.tensor_copy(b1a_rb, b1a_row)
            eps_c = wp.tile([128, 1], F32)
            nc.vector.memset(eps_c, 1e-5)
            zcol = wp.tile([128, 1], F32)
            nc.vector.memset(zcol, 0.0)
            ones_f = wp.tile([128, 128], F32)
            nc.vector.memset(ones_f, 1.0)
            mask_b = wp.tile([128, N], F32)
            mb_src = p_mask[0:1]
            nc.sync.dma_start(
                out=mask_b,
                in_=bass.AP(tensor=mb_src.tensor, offset=0, ap=[[0, 128], [1, N]]),
            )

            # fp8 DR weight tile for layer1: cols 0-511 = 64*[W1r; AF'],
            # cols 512*(r+1).. = 64*NA[r]
            wdr = wp.tile([128, 2048], F8)
            nc.vector.tensor_scalar_mul(wdr[0:64, 0:512], w1r, FS)
            # fp8 DR weights for layer2 (64*W1b, col = k*512+f layout)
            w1b_dr = wp.tile([128, 2048], F8)
            nc.vector.tensor_scalar_mul(w1b_dr, w1b, FS)

            # ================= phase 0: geometry =================
            axs = wp.tile([64, 3], F32)
            nc.vector.tensor_tensor(axs, ax_sb, ax_sb, op=mybir.AluOpType.mult)
            aa2 = wp.tile([64, 1], F32)
            nc.vector.reduce_sum(aa2, axs, axis=mybir.AxisListType.X)
            aug_a = wp.tile([64, 5], F32)
            nc.vector.tensor_scalar_mul(aug_a[:, 0:3], ax_sb, -2.0)
            nc.vector.memset(aug_a[:, 3:4], 1.0)
            nc.vector.tensor_copy(aug_a[:, 4:5], aa2)
            p_t = psA.tile([128, 64], F32, tag="a")
            nc.tensor.transpose(p_t[0:5, 0:64], aug_a, ident[0:64, 0:64])
            axaug = wp.tile([5, 64], F32)
            nc.vector.tensor_copy(axaug, p_t[0:5, 0:64])


            # bf16 weights for the NA / AF' precomputations
            w1nf_b = wp.tile([128, 1024], BF16); nc.vector.tensor_copy(w1nf_b, w1nf)
            w1af_b = wp.tile([128, 1024], BF16); nc.vector.tensor_copy(w1af_b, w1af)

            nfT_b = wp.tile([128, 768], BF16)   # nf.T bf16, kc-major
            nfT_f = wp.tile([128, 768], F32)    # nf.T f32
            # --- batched distance pipeline: all 3 node blocks in one pass ---
            aug_n = wp.tile([128, 15], F32)
            nxs = wp.tile([128, 3], F32)
            for r in range(3):
                nc.vector.tensor_tensor(nxs, nx_sb[r], nx_sb[r], op=mybir.AluOpType.mult)
                nc.vector.reduce_sum(aug_n[:, 5 * r + 3:5 * r + 4], nxs,
                                     axis=mybir.AxisListType.X)
                nc.vector.tensor_copy(aug_n[:, 5 * r:5 * r + 3], nx_sb[r])
                nc.vector.memset(aug_n[:, 5 * r + 4:5 * r + 5], 1.0)
            nxaugT = wp.tile([128, 128], F32)
            axaug3 = wp.tile([128, 64], F32)
            for r in range(3):
                p_tn = psB.tile([128, 128], F32, tag="b", name=f"ptn{r}")
                nc.tensor.transpose(p_tn[0:5, :], aug_n[:, 5 * r:5 * r + 5], ident)
                nc.vector.tensor_copy(nxaugT[32 * r:32 * r + 5, :], p_tn[0:5, :])
                nc.vector.tensor_copy(axaug3[32 * r:32 * r + 5, :], axaug)
            p_d2 = psB.tile([128, 512], F32, tag="b")
            for r in range(3):
                nc.tensor.matmul(p_d2[:, 64 * r:64 * r + 64],
                                 nxaugT[32 * r:32 * r + 5, :],
                                 axaug3[32 * r:32 * r + 5, :],
                                 start=True, stop=True)
            d2c = wp.tile([128, 192], F32)
            nc.vector.tensor_scalar_max(d2c, p_d2[:, 0:192], 0.0)
            t_sb = wp.tile([128, 192], F32)
            nc.scalar.activation(t_sb, d2c, AF.Sqrt, bias=zcol[:, 0:1], scale=D2_SCALE)
            nc.gpsimd.dma_start(t_dram3[:, :], t_sb)

            for r in range(3):
                # nf transpose blocks + NA matmuls
                for c in range(2):
                    p_tr = psB.tile([128, 128], F32, tag="b")
                    nc.tensor.transpose(p_tr, nf_sb[r][:, 128 * c:128 * c + 128], ident)
                    nc.vector.tensor_copy(
                        nfT_b[:, 384 * c + 128 * r:384 * c + 128 * r + 128], p_tr)
                    nc.vector.tensor_copy(
                        nfT_f[:, 384 * c + 128 * r:384 * c + 128 * r + 128], p_tr)
            for r in range(3):
                p_na = psB.tile([128, 512], F32, tag="b")
                for kc in range(2):
                    nc.tensor.matmul(
                        p_na,
                        nfT_b[:, 384 * kc + 128 * r:384 * kc + 128 * r + 128],
                        w1nf_b[:, 512 * kc:512 * kc + 512],
                        start=(kc == 0), stop=(kc == 1),
                    )
                nc.vector.tensor_scalar_mul(
                    wdr[:, 512 * (r + 1):512 * (r + 2)], p_na, FS)

            # af.T and AF' = af@W1a[H:2H] + b1a -> wdr rows 64-127
            afT_b = wp.tile([128, 128], BF16)
            for c in range(2):
                p_tr = psA.tile([128, 64], F32, tag="a")
                nc.tensor.transpose(p_tr[:, 0:64], af_sb[:, 128 * c:128 * c + 128],
                                    ident[0:64, 0:64])
                nc.vector.tensor_copy(afT_b[:, 64 * c:64 * c + 64], p_tr[:, 0:64])
            p_af2 = psB.tile([64, 512], F32, tag="b")
            for kc in range(2):
                nc.tensor.matmul(p_af2, afT_b[:, 64 * kc:64 * kc + 64],
                                 w1af_b[:, 512 * kc:512 * kc + 512],
                                 start=(kc == 0), stop=False)
            nc.tensor.matmul(p_af2, ones64, b1a_rb, start=False, stop=True)
            nc.vector.tensor_scalar_mul(wdr[64:128, 0:512], p_af2, FS)

            sT = [wp.tile([128, 512], BF16, tag=f"sT{r}", name=f"sT{r}")
                  for r in range(3)]  # per-node-block anchor-sums (fc-major x node)

            def ap3(t2d, kstride, kn, inner):
                """[p, kn, inner] AP from a 2D slice (middle dim stride kstride)."""
                return bass.AP(tensor=t2d.tensor, offset=t2d.offset,
                               ap=[list(t2d.ap[0]), [kstride, kn], [1, inner]])

            # ================= main loop (software-pipelined) =================
            def stageA(pp):
                """t gather + rbf for superchunk pair (2pp, 2pp+1)."""
                tb = tbp.tile([128, 1024], F32, tag="tb", name=f"tb{pp}")
                for h in range(2):
                    sc = 2 * pp + h
                    nc.sync.dma_start(
                        out=tb[64 * h:64 * h + 64, :].rearrange(
                            "p (n a) -> p n a", a=64),
                        in_=bass.AP(tensor=t_dram3[:].tensor,
                                    offset=(16 * sc % 128) * 192 + (sc // 8) * 64,
                                    ap=[[0, 64], [192, 16], [1, 64]]),
                    )
                qx = qp.tile([128, 1024], F32, tag="qx", name=f"qx{pp}")
                nc.scalar.activation(qx, tb, AF.Square, bias=negmu[:, 0:1], scale=1.0)
                for h in range(2):
                    c0 = (2 * pp + h) * 2
                    v0 = 8 * ((c0 // 8) % 2) + c0 % 8
                    ov = vt[v0]
                    out_ap = bass.AP(tensor=ov.tensor, offset=ov.offset,
                                     ap=[[ov.ap[0][0], 64], [1024, 2], [1, 512]])
                    nc.scalar.activation(
                        out_ap, qx[64 * h:64 * h + 64, :].rearrange(
                            "p (t x) -> p t x", t=2),
                        AF.Exp, bias=zcol[64 * h:64 * h + 64, 0:1], scale=-1.0)

            def stageB(c):
                """L1 fp8-DR matmuls + relu -> h1 (fp8)."""
                r = c // 16
                v = 8 * ((c // 8) % 2) + c % 8
                rhs = ap3(vt[v][:, 0:512], 512, 2, 512)
                h1 = h1p.tile([128, 2048], F8, tag="h1", name=f"h1_{c}")
                for hh in range(2):
                    p1 = psA.tile([128, 1024], F32, tag="a", name=f"p1_{c}_{hh}")
                    for i in range(2):
                        fc = 2 * hh + i
                        lhsT = ap3(wdr[:, 128 * fc:128 * fc + 128],
                                   512 * (r + 1), 2, 128)
                        nc.tensor.matmul(p1[:, 512 * i:512 * i + 512], lhsT, rhs,
                                         start=True, stop=True, perf_mode=DR)
                    if hh == 0:
                        nc.scalar.activation(h1[:, 0:1024], p1, AF.Relu,
                                             bias=zcol[:, 0:1], scale=1.0 / FS)
                    else:
                        zap = bass.AP(tensor=zcol.tensor, offset=zcol.offset,
                                      ap=[list(zcol.ap[0]), [0, 1024]])
                        with nc.allow_low_precision(reason="fp8 h1 feeds fp8 matmul"):
                            nc.vector.scalar_tensor_tensor(
                                h1[:, 1024:2048], p1, 1.0 / FS, zap,
                                op0=mybir.AluOpType.mult, op1=mybir.AluOpType.max)
                return h1

            h2_live = {}

            def stageC1(c, h1):
                """L2 fp8-DR matmuls + relu (ACT/DVE split) + Pool L1 halving."""
                h2t = h2p.tile([128, 2048], BF16, tag="h2", name=f"h2_{c}")
                for fc in range(4):
                    p2 = psB.tile([128, 512], F32, tag="b", name=f"p2_{c}_{fc}")
                    for kc in range(2):
                        lhsT = ap3(
                            w1b_dr[:, 1024 * kc + 128 * fc:1024 * kc + 128 * fc + 128],
                            512, 2, 128)
                        rhs = ap3(h1[:, 1024 * kc:1024 * kc + 512], 512, 2, 512)
                        nc.tensor.matmul(p2, lhsT, rhs,
                                         start=(kc == 0), stop=(kc == 1),
                                         perf_mode=DR)
                    # h2 kept 64x-scaled (scale folded into layer-3):
                    # ACT for fc<2, DVE (max(z,-64b)+64b == relu(z+64b)) else
                    if fc < 2:
                        nc.scalar.activation(h2t[:, 512 * fc:512 * fc + 512],
                                             p2, AF.Relu,
                                             bias=b1b64_c[:, fc:fc + 1],
                                             scale=1.0)
                    else:
                        bcol = b1b64_c[:, fc:fc + 1]
                        bbc = bass.AP(tensor=bcol.tensor, offset=bcol.offset,
                                      ap=[list(bcol.ap[0]), [0, 512]])
                        with nc.allow_low_precision(reason="bf16 h2 partial sums"):
                            nc.vector.scalar_tensor_tensor(
                                h2t[:, 512 * fc:512 * fc + 512], p2,
                                nb1b64_c[:, fc:fc + 1], bbc,
                                op0=mybir.AluOpType.max,
                                op1=mybir.AluOpType.add)
                # anchor-sum level 1 on Pool (single batched instr)
                rha = redp.tile([128, 1024], BF16, tag="rha", name=f"rha_{c}")
                hv = h2t.rearrange("p (x t a) -> p x t a", t=2, a=32)
                with nc.allow_low_precision(reason="bf16 h2 partial sums"):
                    nc.gpsimd.tensor_tensor(
                        rha.rearrange("p (x a) -> p x a", a=32),
                        hv[:, :, 0, :], hv[:, :, 1, :], op=mybir.AluOpType.add)
                return rha

            def stageC2(c, rha):
                """Anchor-sum: Pool L2 halving + DVE segmented reduce into sT."""
                r = c // 16
                rhb = redp.tile([128, 512], BF16, tag="rhb", name=f"rhb_{c}")
                rv = rha.rearrange("p (x t a) -> p x t a", t=2, a=16)
                with nc.allow_low_precision(reason="bf16 h2 partial sums"):
                    nc.gpsimd.tensor_tensor(
                        rhb.rearrange("p (x a) -> p x a", a=16),
                        rv[:, :, 0, :], rv[:, :, 1, :], op=mybir.AluOpType.add)
                with nc.allow_low_precision(reason="DVE reduce accumulates fp32 internally; bf16 sT feeds bf16 matmul"):
                    nc.vector.tensor_reduce(
                        sT[r].rearrange("p (f n) -> p f n", n=128)[
                            :, 0:4, 8 * (c % 16):8 * (c % 16) + 8],
                        rhb.rearrange("p (f n a) -> p f n a", n=8, a=16),
                        axis=mybir.AxisListType.X,
                        op=mybir.AluOpType.add,
                    )

            # ====== phase 2 (node path), per-128-node block, overlapped ======
            zeros_b = wp.tile([128, 128], BF16)
            nc.vector.memset(zeros_b, 0.0)
            zeros_f = wp.tile([128, 128], F32)
            nc.vector.memset(zeros_f, 0.0)
            p2w = {}

            def load_phase2_weights():
                w1c = wp.tile([128, 1024], F32, name="w1c", tag="w1c")
                dma(w1c.rearrange("p (k f) -> p k f", k=4), p_w1c[:].rearrange("(k p) f -> p k f", p=128))
                w2a = wp.tile([128, 1024], F32, name="w2a", tag="w2a")
                dma(w2a.rearrange("p (k f) -> p k f", k=2), p_w2a[:].rearrange("(k p) f -> p k f", p=128))
                w2b = wp.tile([128, 2048], F32, name="w2b", tag="w2b")
                dma(w2b.rearrange("p (k f) -> p k f", k=4), p_w2b[:].rearrange("(k p) f -> p k f", p=128))
                w2c = wp.tile([128, 1024], F32, name="w2c", tag="w2c")
                dma(w2c.rearrange("p (k f) -> p k f", k=4), p_w2c[:].rearrange("(k p) f -> p k f", p=128))
                for nm, t in (("w1c", w1c), ("w2a", w2a), ("w2b", w2b), ("w2c", w2c)):
                    b = wp.tile(list(t.shape), BF16, name=nm + "_b", tag=nm + "_b")
                    nc.gpsimd.tensor_copy(b, t)
                    p2w[nm] = b

            def block_ln(r, x_in, g_c, b_c, out_t):
                """LN over 256 feats for 128 nodes; x_in/out_t [128, 256] fo-major.
                Cross-partition sums via all-ones fp32 PE matmuls (broadcast to
                all partitions); rstd = exp(-0.5*ln(v)) keeps one ACT table."""
                x3 = x_in.rearrange("p (f n) -> p f n", f=2)
                red = psC.tile([128, 128], F32, tag="c", name=f"lnr{r}")
                nc.tensor.matmul(red, ones_f, x_in[:, 0:128], start=True, stop=False)
                nc.tensor.matmul(red, ones_f, x_in[:, 128:256], start=False, stop=True)
                Sb = bass.AP(tensor=red.tensor, offset=red.offset,
                             ap=[list(red.ap[0]), [0, 2], list(red.ap[1])])
                xc = wp.tile([128, 256], F32, tag=f"lnxc{r}", name=f"lnxc{r}")
                nc.vector.scalar_tensor_tensor(
                    xc.rearrange("p (f n) -> p f n", f=2), Sb, -1.0 / 256.0, x3,
                    op0=mybir.AluOpType.mult, op1=mybir.AluOpType.add)
                sq = wp.tile([128, 256], F32, tag=f"lnsq{r}", name=f"lnsq{r}")
                nc.vector.tensor_tensor(sq, xc, xc, op=mybir.AluOpType.mult)
                red2 = psC.tile([128, 128], F32, tag="c", name=f"lnr2{r}")
                nc.tensor.matmul(red2, ones_f, sq[:, 0:128], start=True, stop=False)
                nc.tensor.matmul(red2, ones_f, sq[:, 128:256], start=False, stop=True)
                lnv = wp.tile([128, 128], F32, tag=f"lnv{r}", name=f"lnv{r}")
                nc.scalar.activation(lnv, red2, AF.Ln, bias=eps_c[:, 0:1],
                                     scale=1.0 / 256.0)
                rstd = wp.tile([128, 128], F32, tag=f"lnrstd{r}", name=f"lnrstd{r}")
                nc.scalar.activation(rstd, lnv, AF.Exp, bias=zcol[:, 0:1], scale=-0.5)
                rb = bass.AP(tensor=rstd.tensor, offset=rstd.offset,
                             ap=[list(rstd.ap[0]), [0, 2], list(rstd.ap[1])])
                y = wp.tile([128, 256], F32, tag=f"lny{r}", name=f"lny{r}")
                nc.vector.tensor_tensor(y.rearrange("p (f n) -> p f n", f=2),
                                        xc.rearrange("p (f n) -> p f n", f=2), rb,
                                        op=mybir.AluOpType.mult)
                for fo in range(2):
                    bcc = b_c[:, fo:fo + 1]
                    bccb = bass.AP(tensor=bcc.tensor, offset=bcc.offset,
                                   ap=[list(bcc.ap[0]), [0, 128]])
                    nc.vector.scalar_tensor_tensor(
                        out_t[:, 128 * fo:128 * fo + 128],
                        y[:, 128 * fo:128 * fo + 128], g_c[:, fo:fo + 1], bccb,
                        op0=mybir.AluOpType.mult, op1=mybir.AluOpType.add)

            p2state = {}

            def phase2_piece(r, piece):
                w1c_b, w2a_b = p2w["w1c"], p2w["w2a"]
                w2b_b, w2c_b = p2w["w2b"], p2w["w2c"]
                psP = psB if r == 2 else psC
                ptag = "b" if r == 2 else "c"
                st = p2state.setdefault(r, {})
                if piece == 0:
                    # L3 + x1 + LN1 (+ bf16 cast)
                    upd = wp.tile([128, 256], F32, tag=f"upd{r}", name=f"upd{r}")
                    for fo in range(2):
                        p3 = psP.tile([128, 128], F32, tag=ptag, name=f"p3_{r}_{fo}")
                        for kc in range(4):
                            nc.tensor.matmul(
                                p3,
                                w1c_b[:, 256 * kc + 128 * fo:256 * kc + 128 * fo + 128],
                                sT[r][:, 128 * kc:128 * kc + 128],
                                start=(kc == 0), stop=(kc == 3),
                            )
                        bcc = b1c_c[:, fo:fo + 1]
                        bccb = bass.AP(tensor=bcc.tensor, offset=bcc.offset,
                                       ap=[list(bcc.ap[0]), [0, 128]])
                        nc.vector.scalar_tensor_tensor(
                            upd[:, 128 * fo:128 * fo + 128], p3,
                            1.0 / (64.0 * 64.0), bccb,
                            op0=mybir.AluOpType.mult, op1=mybir.AluOpType.add)
                    x1 = wp.tile([128, 256], F32, tag=f"x1{r}", name=f"x1{r}")
                    tmpw = wp.tile([128, 256], F32, tag=f"updm{r}", name=f"updm{r}")
                    mb = mask_b[:, 128 * r:128 * r + 128]
                    mb2 = bass.AP(tensor=mb.tensor, offset=mb.offset,
                                  ap=[list(mb.ap[0]), [0, 2], list(mb.ap[1])])
                    nc.vector.tensor_tensor(
                        tmpw.rearrange("p (f n) -> p f n", f=2),
                        upd.rearrange("p (f n) -> p f n", f=2), mb2,
                        op=mybir.AluOpType.mult)
                    nfs = nfT_f[:, 128 * r:128 * r + 128]
                    nfv = bass.AP(tensor=nfs.tensor, offset=nfs.offset,
                                  ap=[list(nfs.ap[0]), [384, 2], list(nfs.ap[1])])
                    nc.vector.tensor_tensor(
                        x1.rearrange("p (f n) -> p f n", f=2),
                        tmpw.rearrange("p (f n) -> p f n", f=2), nfv,
                        op=mybir.AluOpType.add)
                    nf1 = wp.tile([128, 256], F32, tag=f"nf1{r}", name=f"nf1{r}")
                    block_ln(r, x1, ln1g_c, ln1b_c, nf1)
                    nf1_b = wp.tile([128, 256], BF16, tag=f"nf1b{r}", name=f"nf1b{r}")
                    nc.vector.tensor_copy(nf1_b, nf1)
                    st["nf1"], st["nf1_b"] = nf1, nf1_b
                elif piece == 1:
                    nf1_b = st["nf1_b"]
                    g1 = wp.tile([128, 512], BF16, tag=f"g1{r}", name=f"g1{r}")
                    for fc in range(4):
                        p = psP.tile([128, 128], F32, tag=ptag, name=f"pg1_{r}_{fc}")
                        for kc in range(2):
                            nc.tensor.matmul(
                                p, w2a_b[:, 512 * kc + 128 * fc:512 * kc + 128 * fc + 128],
                                nf1_b[:, 128 * kc:128 * kc + 128],
                                start=(kc == 0), stop=(kc == 1))
                        nc.vector.scalar_tensor_tensor(
                            g1[:, 128 * fc:128 * fc + 128], p, b2a_c[:, fc:fc + 1],
                            zeros_b, op0=mybir.AluOpType.add, op1=mybir.AluOpType.max)
                    st["g1"] = g1
                elif piece == 2:
                    g1 = st["g1"]
                    g2 = wp.tile([128, 512], BF16, tag=f"g2{r}", name=f"g2{r}")
                    for fc in range(4):
                        p = psP.tile([128, 128], F32, tag=ptag, name=f"pg2_{r}_{fc}")
                        for kc in range(4):
                            nc.tensor.matmul(
                                p, w2b_b[:, 512 * kc + 128 * fc:512 * kc + 128 * fc + 128],
                                g1[:, 128 * kc:128 * kc + 128],
                                start=(kc == 0), stop=(kc == 3))
                        nc.vector.scalar_tensor_tensor(
                            g2[:, 128 * fc:128 * fc + 128], p, b2b_c[:, fc:fc + 1],
                            zeros_b, op0=mybir.AluOpType.add, op1=mybir.AluOpType.max)
                    st["g2"] = g2
                elif piece == 3:
                    g2, nf1 = st["g2"], st["nf1"]
                    upd2 = wp.tile([128, 256], F32, tag=f"upd2{r}", name=f"upd2{r}")
                    for fo in range(2):
                        p = psP.tile([128, 128], F32, tag=ptag, name=f"pu2_{r}_{fo}")
                        for kc in range(4):
                            nc.tensor.matmul(
                                p, w2c_b[:, 256 * kc + 128 * fo:256 * kc + 128 * fo + 128],
                                g2[:, 128 * kc:128 * kc + 128],
                                start=(kc == 0), stop=(kc == 3))
                        nc.vector.scalar_tensor_tensor(
                            upd2[:, 128 * fo:128 * fo + 128], p, b2c_c[:, fo:fo + 1],
                            zeros_f, op0=mybir.AluOpType.add, op1=mybir.AluOpType.add)
                    x2 = wp.tile([128, 256], F32, tag=f"x2{r}", name=f"x2{r}")
                    tmpw2 = wp.tile([128, 256], F32, tag=f"updm2{r}", name=f"updm2{r}")
                    mb = mask_b[:, 128 * r:128 * r + 128]
                    mb2 = bass.AP(tensor=mb.tensor, offset=mb.offset,
                                  ap=[list(mb.ap[0]), [0, 2], list(mb.ap[1])])
                    nc.vector.tensor_tensor(
                        tmpw2.rearrange("p (f n) -> p f n", f=2),
                        upd2.rearrange("p (f n) -> p f n", f=2), mb2,
                        op=mybir.AluOpType.mult)
                    nc.vector.tensor_tensor(x2, tmpw2, nf1, op=mybir.AluOpType.add)
                    outT = wp.tile([128, 256], F32, tag=f"outT{r}", name=f"outT{r}")
                    block_ln(r, x2, ln2g_c, ln2b_c, outT)
                    st["outT"] = outT
                else:
                    outT = st["outT"]
                    o_sb = outp.tile([128, 256], F32, tag="osb", name=f"osb{r}")
                    for fo in range(2):
                        p_tr = psP.tile([128, 128], F32, tag=ptag, name=f"ptr_{r}_{fo}")
                        nc.tensor.transpose(p_tr, outT[:, 128 * fo:128 * fo + 128], ident)
                        nc.vector.tensor_copy(o_sb[:, 128 * fo:128 * fo + 128], p_tr)
                    nc.sync.dma_start(p_out[128 * r:128 * r + 128, :], o_sb)

            for _pp in range(2):
                stageA(_pp)
            h1_live = {}
            for c in range(NC_CHUNKS + 5):
                if c == 8:
                    load_phase2_weights()
                if 0 <= c - 4 < NC_CHUNKS:
                    stageC2(c - 4, h2_live.pop(c - 4))
                if 0 <= c - 3 < NC_CHUNKS:
                    h2_live[c - 3] = stageC1(c - 3, h1_live.pop(c - 3))
                if c % 4 == 0 and c // 4 + 2 < 12:
                    stageA(c // 4 + 2)
                if c < NC_CHUNKS:
                    h1_live[c] = stageB(c)
                for rr, base in ((0, 21), (1, 37)):
                    if base <= c < base + 10 and (c - base) % 2 == 0:
                        phase2_piece(rr, (c - base) // 2)
                if c == 52:
                    for pp in range(5):
                        phase2_piece(2, pp)

    nc.compile()
    return nc


_NC = None
_CONSTS = _consts()


def build_in_maps(inputs):
    shared = {k: np.ascontiguousarray(np.asarray(inputs[k], np.float32))
              for k in ("W1a", "b1a", "W1b", "b1b", "W1c", "b1c", "ln1_g", "ln1_b",
                        "W2a", "b2a", "W2b", "b2b", "W2c", "b2c", "ln2_g", "ln2_b")}
    shared.update(_CONSTS)
    shared["c_biases"] = _pack_biases(inputs)

    node_x = np.asarray(inputs["node_x"], np.float32)
    anchor_x = np.asarray(inputs["anchor_x"], np.float32)
    node_features = np.asarray(inputs["node_features"], np.float32)
    anchor_features = np.asarray(inputs["anchor_features"], np.float32)
    node_mask = np.asarray(inputs["node_mask"], np.float32)

    in_maps = []
    for b in range(B):
        m = dict(shared)
        m["node_x"] = np.ascontiguousarray(node_x[b * N:(b + 1) * N])
        m["anchor_x"] = np.ascontiguousarray(anchor_x[b * A:(b + 1) * A])
        m["node_features"] = np.ascontiguousarray(node_features[b * N:(b + 1) * N])
        m["anchor_features"] = np.ascontiguousarray(
            anchor_features[b * A:(b + 1) * A])
        m["node_mask"] = np.ascontiguousarray(node_mask[b * N:(b + 1) * N])
        in_maps.append(m)
    return in_maps


def kernel(**inputs):
    global _NC
    if _NC is None:
        _NC = _build()
    in_maps = build_in_maps(inputs)
    res = run_bass_kernel_spmd(_NC, in_maps, core_ids=list(range(B)))
    return np.concatenate([res.results[b]["out"] for b in range(B)], axis=0)
